# revision 47
# baseline (speedup 1.0000x reference)
"""LogSparseAttention Trainium2 kernel (8-core SPMD), fp16 pipelined version.

Sharding: 8 cores = 2 batches x 4 head-groups (4 heads = 256 channels each).
Per core: causal convs (q, k) over full-D input for its 256 channels, v
projection, local window-16 attention + 8 exponential-jump terms for its 4
heads, partial output projection over its 256 channels. Host sums the 8
partial [D, T] outputs (4 per batch) and adds p_b.

v2 changes vs baseline (838us -> 653us):
  - fp16 matmuls everywhere (convs/scores/po/jump/proj); bf16 for exp tiles
    and v_td. fp16 streams at the same 1 cyc/row as f32r for big matmuls but
    kills the f32r small-AP 4x penalty (66-col matmuls: 160ns -> 58ns) and
    halves DMA/SBUF. End-to-end accuracy 2.1e-3 vs the 2e-2 gate.
  - conv weights repacked host-side to [g, ktile, 128, (tap, out-ch)] so each
    weight DMA is one 512KB transfer with 4KB lines feeding 16 matmuls (the
    naive per-(tap,ktile) 32KB/256B-line stream starved the PE).
  - conv runs as 2 pair-passes per head-group (q-pass then k-pass per 1024
    tokens) using only 2 PSUM banks, leaving 6 for the concurrent attention.
  - per-head-group software pipeline: g0's attention/jump work (DVE/ACT/
    GpSimd/DMA) executes under g1's conv PE time; local-attention chunks and
    jump-score quarters are interleaved into the conv chunk stream as their
    k-ranges become available.
  - jump FMA = alpha_e (*) v-shifted, accumulated elementwise on two engine
    lanes (DVE + GpSimd) into separate zj accumulators; DVE lanes read
    shift-aligned v copies built by SBUF->SBUF DMA (non-256B-aligned f16
    DVE ops run ~4x slow); no serial DMA-accumulate chain.
  - DMA queue routing to avoid head-of-line blocking: g0's fma DMAs on the
    ACT queue (sync carries the conv weight stream), g1's on sync (idle in
    the tail, keeps ACT clear for local-attn exps).

Layouts on chip (partition dim first):
  xT      8 x [128, 2063] f16     x transposed, left-padded 15 zeros
  q_sb    2 x [128, 2048] f16     conv out (+bias, x1/8)
  k_sb    2 x [128, 2063] f16     conv out (+bias), left-padded zeros
  v_dt    2 x [128, 2048] f16     v (+bias), channel-partition (jump FMA)
  v_td    17 x [128, 4, 66] bf16  v (+bias) token-partition, +15-shifted,
                                  per-head ones column (Z accumulator)
  z       2 x [128, 2048] f16     local attention output [d, t]
  zj      2 x [128, 2048] f16     jump-term accumulators (shared across g)
"""
import sys

sys.path.insert(0, "/opt/trn_rl_repo")

import numpy as np
import ml_dtypes
import concourse.bass as bass
import concourse.bacc as bacc
import concourse.tile as tile
from concourse import mybir

f32 = mybir.dt.float32
f16 = mybir.dt.float16
bf16 = mybir.dt.bfloat16
AL = mybir.AluOpType
AF = mybir.ActivationFunctionType

B, T, D = 2, 2048, 1024
H, W, E = 16, 16, 8
HD = D // H                  # 64
NCORES = 8
HPC = 4                      # heads per core
CH = HPC * HD                # 256 channels per core
NG = 2                       # head-pairs per core, 128 channels each
TP = W - 1                   # 15 (left pad)
TPAD = T + TP                # 2063
NT128 = T // 128             # 16
NT256 = T // 256             # 8
NT512 = T // 512             # 4
KT = D // 128                # 8 k-tiles over the input dim
SCALE = 1.0 / float(np.sqrt(HD))
MASKVAL = -200.0

_CACHE = {}


def build_program():
    if "nc" in _CACHE:
        return _CACHE["nc"]
    import contextlib
    nc = bacc.Bacc()

    xT = nc.dram_tensor("xT", [D, TPAD], f16, kind="ExternalInput")
    # weights packed [g, ktile, in-ch-in-tile, (tap, out-ch)] for 4KB DMA lines
    qw = nc.dram_tensor("qw", [NG, KT, 128, W * 128], f16, kind="ExternalInput")
    kw = nc.dram_tensor("kw", [NG, KT, 128, W * 128], f16, kind="ExternalInput")
    vw = nc.dram_tensor("vw", [D, CH], f16, kind="ExternalInput")
    pw = nc.dram_tensor("pw", [CH, D], f16, kind="ExternalInput")
    qb = nc.dram_tensor("qb", [CH, 1], f32, kind="ExternalInput")
    kb = nc.dram_tensor("kb", [CH, 1], f32, kind="ExternalInput")
    vb = nc.dram_tensor("vb", [CH, 1], f32, kind="ExternalInput")
    mask = nc.dram_tensor("mask", [272, 256], f32, kind="ExternalInput")
    ident = nc.dram_tensor("ident", [128, 128], f16, kind="ExternalInput")
    onesp = nc.dram_tensor("onesp", [E, 128, 2 * E], f16, kind="ExternalInput")
    ones4 = nc.dram_tensor("ones4", [128, 2 * HPC], bf16, kind="ExternalInput")
    zpad = nc.dram_tensor("zpad", [128, TP], f16, kind="ExternalInput")
    vbrow = nc.dram_tensor("vbrow", [1, CH], bf16, kind="ExternalInput")
    vzero = nc.dram_tensor("vzero", [TP, CH], bf16, kind="ExternalInput")
    y = nc.dram_tensor("y", [D, T], f16, kind="ExternalOutput")
    alpha_d = [nc.dram_tensor(f"alpha_d{g}", [2 * E, T], f16) for g in range(NG)]

    with tile.TileContext(nc) as tc:
        with contextlib.ExitStack() as ctx:
            consts = ctx.enter_context(tc.tile_pool(name="consts", bufs=1))
            main = ctx.enter_context(tc.tile_pool(name="main", bufs=1))

            # ---- x and v-weights first: the PE's first work needs them ----
            xT_sb = [main.tile([128, TPAD], f16, tag=f"x{i}", name=f"xT_sb{i}") for i in range(KT)]
            vw_sb = [consts.tile([128, CH], f16, tag=f"vw{i}", name=f"vw_sb{i}") for i in range(KT)]
            for i in range(KT):
                if i % 2 == 0:
                    nc.sync.dma_start(xT_sb[i][:], xT[128 * i:128 * (i + 1), :])
                else:
                    nc.scalar.dma_start(xT_sb[i][:], xT[128 * i:128 * (i + 1), :])
                nc.sync.dma_start(vw_sb[i][:], vw[128 * i:128 * (i + 1), :])
            vbt = consts.tile([128, CH], bf16)
            nc.sync.dma_start(vbt[:], vbrow[:].to_broadcast((128, CH)))

            # ---- remaining constants ----
            m0 = consts.tile([128, 256], f32)
            m1 = consts.tile([128, 256], f32)
            m2 = consts.tile([TP, 256], f32)
            nc.sync.dma_start(m0[:], mask[0:128, :])
            nc.sync.dma_start(m1[:], mask[128:256, :])
            nc.sync.dma_start(m2[:], mask[256:271, :])
            id_sb = consts.tile([128, 128], f16)
            nc.sync.dma_start(id_sb[:], ident[:])
            onesp_sb = consts.tile([128, E, 2 * E], f16)
            nc.sync.dma_start(onesp_sb[:], onesp.rearrange("e p m -> p e m"))
            qb_sb = consts.tile([128, NG], f32)
            kb_sb = consts.tile([128, NG], f32)
            vb_sb = consts.tile([128, NG], f32)
            nc.sync.dma_start(qb_sb[:], qb.rearrange("(g p) o -> p (g o)", g=NG))
            nc.sync.dma_start(kb_sb[:], kb.rearrange("(g p) o -> p (g o)", g=NG))
            nc.sync.dma_start(vb_sb[:], vb.rearrange("(g p) o -> p (g o)", g=NG))
            pw_sb = [consts.tile([128, D], f16, tag=f"pw{g}", name=f"pw_sb{g}") for g in range(NG)]
            for g in range(NG):
                nc.sync.dma_start(pw_sb[g][:], pw[128 * g:128 * (g + 1), :])
            q_sb = [main.tile([128, T], f16, tag=f"q{g}", name=f"q_sb{g}") for g in range(NG)]
            k_sb = [main.tile([128, TPAD], f16, tag=f"k{g}", name=f"k_sb{g}") for g in range(NG)]
            v_dt = [main.tile([128, T], f16, tag=f"vdt{g}", name=f"v_dt{g}") for g in range(NG)]
            v_td = [main.tile([128, HPC, HD + 2], bf16, tag=f"vtd{j}", name=f"v_td{j}")
                    for j in range(NT128 + 1)]
            z = [main.tile([128, T], f16, tag=f"z{g}", name=f"z{g}") for g in range(NG)]
            zj_sh = [main.tile([128, T], f16, tag=f"zj{a}", name=f"zj{a}")
                     for a in range(2)]
            zj = [zj_sh, zj_sh]          # per-g lifetimes do not overlap
            zr = [main.tile([128, T], f16, tag=f"zr{g}", name=f"zr{g}") for g in range(NG)]
            arows_sh = main.tile([2 * E, T], f16, tag="ar", name="arows")
            arows = [arows_sh, arows_sh]

            for g in range(NG):
                nc.sync.dma_start(k_sb[g][:, 0:TP], zpad[:])

            # ================= phase 1: v projections =================
            with tc.tile_pool(name="psV", bufs=1, space="PSUM") as psV:
                # v in [t, d] layout (M=t), shifted +15, with bias
                for j in range(NT128 + 1):
                    mrow = 128 if j < NT128 else TP
                    pv = psV.tile([128, 256], f32, tag=f"pv{j % 2}", bufs=1, name=f"pv{j}")
                    for i in range(KT):
                        nc.tensor.matmul(
                            pv[0:mrow, :],
                            xT_sb[i][:, 128 * j:128 * j + mrow],
                            vw_sb[i][:],
                            start=(i == 0), stop=(i == KT - 1),
                        )
                    nc.vector.tensor_tensor(
                        v_td[j][0:mrow, :, 0:HD],
                        pv[0:mrow, :].rearrange("p (h d) -> p h d", h=HPC),
                        vbt[0:mrow, :].rearrange("p (h d) -> p h d", h=HPC),
                        AL.add,
                    )
                    if j == 0:
                        # keys/values at t<0 are zero AFTER bias in the reference
                        nc.sync.dma_start(
                            v_td[0][0:TP, :, 0:HD],
                            vzero.rearrange("p (h d) -> p h d", h=HPC))
                    nc.sync.dma_start(
                        v_td[j][:, :, HD:HD + 2],
                        ones4.rearrange("p (h o) -> p h o", o=2))

                # v in [d, t] layout (M=d), with bias
                for g in range(NG):
                    for t4 in range(NT512):
                        pv2 = psV.tile([128, 512], f32, tag=f"pv2{t4 % 2}", bufs=1,
                                       name=f"pv2_{g}_{t4}")
                        for i in range(KT):
                            nc.tensor.matmul(
                                pv2[:],
                                vw_sb[i][:, 128 * g:128 * (g + 1)],
                                xT_sb[i][:, TP + 512 * t4: TP + 512 * (t4 + 1)],
                                start=(i == 0), stop=(i == KT - 1),
                            )
                        nc.vector.tensor_scalar(
                            v_dt[g][:, 512 * t4:512 * (t4 + 1)], pv2[:],
                            vb_sb[:, g:g + 1], None, op0=AL.add,
                        )

            # ======= phases 2-3: per-g conv then attention, pipelined =======
            with tc.tile_pool(name="attn", bufs=1) as apool, \
                 tc.tile_pool(name="wstream", bufs=6) as wpool, \
                 tc.tile_pool(name="psC", bufs=1, space="PSUM") as psC, \
                 tc.tile_pool(name="psA", bufs=1, space="PSUM") as psA:

                erows_sh = apool.tile([2 * E, T], bf16, tag="erows",
                                      name="erows")
                erows = [erows_sh, erows_sh]

                def conv_pair(g, p, between=None):
                    # one weight stream covers two 512-token chunks; 2 PSUM
                    # banks total.  Pair 1 runs k first so `between` work
                    # needing only k (jump quarter 1) can overlap the q-pass.
                    passes = ((qw, True), (kw, False)) if p == 0 else \
                             ((kw, False), (qw, True))
                    for wdram, is_q in passes:
                        pcs = [psC.tile([128, 512], f32, tag=f"cv{c}", bufs=1,
                                        name=f"pc{g}_{p}_{c}") for c in range(2)]
                        for i in range(KT):
                            wt = wpool.tile([128, W * 128], f16,
                                            tag=f"w{int(is_q)}", bufs=3)
                            nc.sync.dma_start(wt[:], wdram[g, i])
                            for dt in range(W):
                                first = (i == 0 and dt == 0)
                                last = (i == KT - 1 and dt == W - 1)
                                for c in range(2):
                                    c0 = 512 * (2 * p + c) + dt
                                    nc.tensor.matmul(
                                        pcs[c][:], wt[:, 128 * dt:128 * (dt + 1)],
                                        xT_sb[i][:, c0:c0 + 512],
                                        start=first, stop=last)
                        for c in range(2):
                            t0 = 512 * (2 * p + c)
                            if is_q:
                                nc.vector.tensor_scalar(
                                    q_sb[g][:, t0:t0 + 512], pcs[c][:],
                                    qb_sb[:, g:g + 1], SCALE,
                                    op0=AL.add, op1=AL.mult)
                            else:
                                nc.vector.tensor_scalar(
                                    k_sb[g][:, TP + t0:TP + t0 + 512], pcs[c][:],
                                    kb_sb[:, g:g + 1], None, op0=AL.add)
                        if between is not None:
                            between()
                            between = None

                def jump_quarter(g, q4):
                    pj = psA.tile([2 * E, 512], f32, tag="pj", bufs=1, name="pj")
                    for e in range(E):
                        sh = 1 << e
                        t0 = 512 * q4
                        tmp = apool.tile([128, 512], f16, tag="jtmp", bufs=3,
                                         name="jtmp")
                        ov = max(0, t0 + 512 + sh - T)  # wrap amount
                        nc.vector.tensor_tensor(
                            tmp[:, 0:512 - ov],
                            q_sb[g][:, t0:t0 + 512 - ov],
                            k_sb[g][:, TP + t0 + sh:TP + t0 + sh + 512 - ov],
                            AL.mult)
                        if ov:
                            nc.vector.tensor_tensor(
                                tmp[:, 512 - ov:512],
                                q_sb[g][:, t0 + 512 - ov:t0 + 512],
                                k_sb[g][:, TP:TP + ov], AL.mult)
                        nc.tensor.matmul(
                            pj[:], onesp_sb[:, e, :], tmp[:],
                            start=(e == 0), stop=(e == E - 1))
                    nc.scalar.activation(
                        erows[g][:, 512 * q4:512 * (q4 + 1)], pj[:], AF.Exp)

                def jump_finish(g):
                    zsum = apool.tile([2 * E, 1], f32, tag="zsum", name="zsum")
                    nc.vector.tensor_reduce(
                        zsum[:], erows[g][:], mybir.AxisListType.X, AL.add)
                    nc.vector.reciprocal(zsum[:], zsum[:])
                    nc.vector.tensor_scalar(
                        arows[g][:], erows[g][:], zsum[:], None, op0=AL.mult)
                    nc.sync.dma_start(alpha_d[g][:], arows[g][:])

                def jump_fma(g):
                    # alpha_e * v_shifted accumulated elementwise on two
                    # engine lanes.  DVE/GpSimd f16 ops with non-256B-aligned
                    # slices run ~4x slow, so shifted v copies are built with
                    # SBUF->SBUF DMA and all engine ops are full-width.
                    # g0's FMA runs under conv g1: route it to GpSimd so the
                    # DVE queue stays clear for conv PSUM drains.  g1's FMA is
                    # the tail: DVE-heavy.
                    # GpSimd runs f16 elementwise at ~5us/op whether aligned
                    # or not, so only DVE lanes get the shift-aligned v copy.
                    gp_lanes = (0, 1, 2, 3, 4, 5, 6, 7) if g == 0 else (1, 3)
                    acc = zj[g]          # [lane0 accumulator, lane1 accumulator]
                    first = [True, True]
                    # g0's FMA DMAs go on the ACT queue (sync carries conv g1's
                    # weight stream); g1's go on sync (idle in the tail) so the
                    # ACT queue stays clear for local-attention exps.
                    dma_eng = nc.scalar if g == 0 else nc.sync
                    for e in range(E):
                        sh = 1 << e
                        lane = e % 2
                        on_gp = e in gp_lanes
                        eng = nc.gpsimd if on_gp else nc.vector
                        bc = apool.tile([128, T], f16, tag="bc", bufs=3, name="bc")
                        dma_eng.dma_start(
                            bc[0:64, :],
                            alpha_d[g][2 * e:2 * e + 1, :].to_broadcast((64, T)))
                        dma_eng.dma_start(
                            bc[64:128, :],
                            alpha_d[g][2 * e + 1:2 * e + 2, :].to_broadcast((64, T)))
                        aligned = not (sh % 128) or on_gp
                        if not aligned:
                            vsh = apool.tile([128, T], f16, tag="vsh", bufs=3,
                                             name="vsh")
                            dma_eng.dma_start(vsh[:, 0:T - sh], v_dt[g][:, sh:T])
                            dma_eng.dma_start(vsh[:, T - sh:T], v_dt[g][:, 0:sh])
                        if first[lane]:
                            ft = acc[lane]
                        else:
                            ft = apool.tile([128, T], f16, tag=f"ft{lane}",
                                            bufs=2, name="ft")
                        if not aligned:
                            eng.tensor_tensor(ft[:], bc[:], vsh[:], AL.mult)
                        else:
                            eng.tensor_tensor(
                                ft[:, 0:T - sh], bc[:, 0:T - sh],
                                v_dt[g][:, sh:T], AL.mult)
                            eng.tensor_tensor(
                                ft[:, T - sh:T], bc[:, T - sh:T],
                                v_dt[g][:, 0:sh], AL.mult)
                        if first[lane]:
                            first[lane] = False
                        else:
                            eng.tensor_tensor(
                                acc[lane][:], acc[lane][:], ft[:], AL.add)

                def scores_part(g, c):
                    exps = []
                    for hh in range(2):
                        r0, r1 = 64 * hh, 64 * hh + 64
                        e0 = apool.tile([128, 256], bf16, tag="e0", bufs=8, name="e0")
                        e1 = apool.tile([128, 256], bf16, tag="e1", bufs=8, name="e1")
                        e2 = apool.tile([TP, 256], bf16, tag="e2", bufs=8, name="e2")
                        for (et, msk, s0, srows) in (
                            (e0, m0, 256 * c, 128),
                            (e1, m1, 256 * c + 128, 128),
                            (e2, m2, 256 * c + 256, TP),
                        ):
                            ps = psA.tile([128, 256], f32, tag="score", bufs=2, name="ps")
                            nc.tensor.matmul(
                                ps[0:srows, :],
                                k_sb[g][r0:r1, s0:s0 + srows],
                                q_sb[g][r0:r1, 256 * c:256 * (c + 1)],
                                start=True, stop=True,
                            )
                            nc.vector.tensor_tensor(
                                ps[0:srows, :], ps[0:srows, :],
                                msk[0:srows, :], AL.add)
                            nc.scalar.activation(
                                et[0:srows, :], ps[0:srows, :], AF.Exp)
                        exps.append((e0, e1, e2))
                    return exps

                def po_part(g, c, exps):
                    for sub in range(2):
                            jj = 2 * c + sub
                            stage = apool.tile([128, 128], f16, tag="stage", bufs=2, name="stage")
                            for hh in range(2):
                                e0, e1, e2 = exps[hh]
                                if sub == 0:
                                    lo, hi = e0[:, 0:128], e1[0:TP, 0:128]
                                else:
                                    lo, hi = e1[:, 128:256], e2[0:TP, 128:256]
                                hl = 2 * g + hh
                                po = psA.tile([128, HD + 2], f32, tag="popt", bufs=2, name="po")
                                nc.tensor.matmul(
                                    po[:], lo, v_td[jj][:, hl, :],
                                    start=True, stop=False,
                                )
                                nc.tensor.matmul(
                                    po[:], hi, v_td[jj + 1][0:TP, hl, :],
                                    start=False, stop=True,
                                )
                                rz = apool.tile([128, 1], f32, tag="rz", bufs=2, name="rz")
                                nc.vector.reciprocal(rz[:], po[:, HD:HD + 1])
                                nc.vector.tensor_scalar(
                                    stage[:, 64 * hh:64 * hh + 64],
                                    po[:, 0:HD], rz[:], None, op0=AL.mult,
                                )
                            pt = psA.tile([128, 128], f16, tag="popt", bufs=2, name="pt")
                            nc.tensor.transpose(pt[:], stage[:], id_sb[:])
                            tcol = 256 * c + 128 * sub
                            nc.vector.tensor_copy(z[g][:, tcol:tcol + 128], pt[:])

                def local_attn(g, chunks):
                    # all score matmuls first: the PE streams them back-to-back
                    # while ACT computes the exps, so no po matmul reaches the
                    # PE queue head before its exp inputs are ready
                    staged = [(c, scores_part(g, c)) for c in chunks]
                    for c, exps in staged:
                        po_part(g, c, exps)

                for g in range(NG):
                    conv_pair(g, 0)              # tokens 0..1023
                    local_attn(g, [0, 1, 2, 3])  # keys <= t1023 available
                    jump_quarter(g, 0)           # k up to t640 available
                    conv_pair(g, 1,              # tokens 1024..2047
                              between=lambda g=g: jump_quarter(g, 1))
                    jump_quarter(g, 2)
                    jump_quarter(g, 3)
                    jump_finish(g)
                    if g == 0:
                        # nothing here is tail-critical; keep the ACT queue
                        # clear of alpha-gated bc issues ahead of local exps
                        local_attn(g, [4, 5, 6, 7])
                        jump_fma(g)
                    else:
                        jump_fma(g)
                        local_attn(g, [4, 5, 6, 7])
                    nc.vector.tensor_tensor(zr[g][:], z[g][:], zj[g][0][:], AL.add)
                    nc.vector.tensor_tensor(zr[g][:], zr[g][:], zj[g][1][:], AL.add)

            # ================= phase 4: output projection =================
            with tc.tile_pool(name="proj", bufs=1) as ppool, \
                 tc.tile_pool(name="psP", bufs=1, space="PSUM") as psP:
                for o8 in range(D // 128):
                    for t4 in range(NT512):
                        py = psP.tile([128, 512], f32, tag="py", bufs=4, name="py")
                        for g in range(NG):
                            nc.tensor.matmul(
                                py[:],
                                pw_sb[g][:, 128 * o8:128 * (o8 + 1)],
                                zr[g][:, 512 * t4:512 * (t4 + 1)],
                                start=(g == 0), stop=(g == NG - 1),
                            )
                        ysb = ppool.tile([128, 512], f16, tag="ysb", bufs=4, name="ysb")
                        nc.vector.tensor_copy(ysb[:], py[:])
                        eng = nc.sync if (o8 * NT512 + t4) % 2 == 0 else nc.scalar
                        eng.dma_start(
                            y[128 * o8:128 * (o8 + 1),
                              512 * t4:512 * (t4 + 1)], ysb[:])

    nc.compile()
    _CACHE["nc"] = nc
    return nc


def make_consts():
    mask = np.full((272, 256), MASKVAL, np.float32)
    rel = np.arange(271)[:, None]
    trel = np.arange(256)[None, :]
    band = (rel >= trel) & (rel <= trel + TP)
    mask[:271][band] = 0.0
    ident = np.eye(128, dtype=np.float16)
    onesp = np.zeros((E, 128, 2 * E), np.float16)
    for e in range(E):
        onesp[e, 0:64, 2 * e] = 1.0
        onesp[e, 64:128, 2 * e + 1] = 1.0
    ones4 = np.zeros((128, 2 * HPC), ml_dtypes.bfloat16)
    ones4[:, 0::2] = 1.0
    zpad = np.zeros((128, TP), np.float16)
    return mask, ident, onesp, ones4, zpad


def _pack_conv_w(wslice):
    # wslice [CH, D, W] (torch layout for this core's channels) ->
    # [NG, KT, 128, W*128]: [g, i, in-ch p, (tap dt, out-ch o)]
    a = wslice.reshape(NG, 128, KT, 128, W)         # [g, o, i, p, dt]
    a = a.transpose(0, 2, 3, 4, 1)                  # [g, i, p, dt, o]
    return np.ascontiguousarray(
        a.reshape(NG, KT, 128, W * 128).astype(np.float16))


def make_in_maps(x, q_w, q_b, k_w, k_b, v_w, v_b, p_w):
    mask, ident, onesp, ones4, zpad = make_consts()
    in_maps = []
    for core in range(NCORES):
        b, g = core // HPC, core % HPC
        ch = slice(CH * g, CH * (g + 1))
        xTf = np.zeros((D, TPAD), np.float16)
        xTf[:, TP:] = x[b].T.astype(np.float16)
        in_maps.append({
            "xT": xTf,
            "qw": _pack_conv_w(q_w[ch]),
            "kw": _pack_conv_w(k_w[ch]),
            "vw": np.ascontiguousarray(v_w[ch].T.astype(np.float16)),
            "pw": np.ascontiguousarray(p_w[:, ch].T.astype(np.float16)),
            "qb": np.ascontiguousarray(q_b[ch][:, None].astype(np.float32)),
            "kb": np.ascontiguousarray(k_b[ch][:, None].astype(np.float32)),
            "vb": np.ascontiguousarray(v_b[ch][:, None].astype(np.float32)),
            "mask": mask, "ident": ident, "onesp": onesp,
            "ones4": ones4, "zpad": zpad,
            "vbrow": np.ascontiguousarray(
                v_b[ch][None, :].astype(ml_dtypes.bfloat16)),
            "vzero": np.zeros((TP, CH), ml_dtypes.bfloat16),
        })
    return in_maps


def assemble_output(results, p_b):
    out = np.zeros((B, T, D), np.float32)
    for core in range(NCORES):
        out[core // HPC] += results[core]["y"].T.astype(np.float32)
    out += p_b[None, None, :]
    return out


def _run(inputs, trace=False):
    from concourse.bass_utils import run_bass_kernel_spmd
    nc = build_program()
    args = {k: np.asarray(v, np.float32) for k, v in inputs.items()}
    p_b = args.pop("p_b")
    in_maps = make_in_maps(**args)
    res = run_bass_kernel_spmd(nc, in_maps, list(range(NCORES)), trace=trace)
    out = assemble_output(res.results, p_b)
    return out, res


def kernel(**inputs):
    out, _ = _run(inputs)
    return out


# revision 48
# speedup vs baseline: 1.0154x; 1.0154x over previous
"""LogSparseAttention Trainium2 kernel (8-core SPMD), fp16 pipelined version.

Sharding: 8 cores = 2 batches x 4 head-groups (4 heads = 256 channels each).
Per core: causal convs (q, k) over full-D input for its 256 channels, v
projection, local window-16 attention + 8 exponential-jump terms for its 4
heads, partial output projection over its 256 channels. Host sums the 8
partial [D, T] outputs (4 per batch) and adds p_b.

v2 changes vs baseline (838us -> 653us):
  - fp16 matmuls everywhere (convs/scores/po/jump/proj); bf16 for exp tiles
    and v_td. fp16 streams at the same 1 cyc/row as f32r for big matmuls but
    kills the f32r small-AP 4x penalty (66-col matmuls: 160ns -> 58ns) and
    halves DMA/SBUF. End-to-end accuracy 2.1e-3 vs the 2e-2 gate.
  - conv weights repacked host-side to [g, ktile, 128, (tap, out-ch)] so each
    weight DMA is one 512KB transfer with 4KB lines feeding 16 matmuls (the
    naive per-(tap,ktile) 32KB/256B-line stream starved the PE).
  - conv runs as 2 pair-passes per head-group (q-pass then k-pass per 1024
    tokens) using only 2 PSUM banks, leaving 6 for the concurrent attention.
  - per-head-group software pipeline: g0's attention/jump work (DVE/ACT/
    GpSimd/DMA) executes under g1's conv PE time; local-attention chunks and
    jump-score quarters are interleaved into the conv chunk stream as their
    k-ranges become available.
  - jump FMA = alpha_e (*) v-shifted, accumulated elementwise on two engine
    lanes (DVE + GpSimd) into separate zj accumulators; DVE lanes read
    shift-aligned v copies built by SBUF->SBUF DMA (non-256B-aligned f16
    DVE ops run ~4x slow); no serial DMA-accumulate chain.
  - DMA queue routing to avoid head-of-line blocking: g0's fma DMAs on the
    ACT queue (sync carries the conv weight stream), g1's on sync (idle in
    the tail, keeps ACT clear for local-attn exps).

Layouts on chip (partition dim first):
  xT      8 x [128, 2063] f16     x transposed, left-padded 15 zeros
  q_sb    2 x [128, 2048] f16     conv out (+bias, x1/8)
  k_sb    2 x [128, 2063] f16     conv out (+bias), left-padded zeros
  v_dt    2 x [128, 2048] f16     v (+bias), channel-partition (jump FMA)
  v_td    17 x [128, 4, 66] bf16  v (+bias) token-partition, +15-shifted,
                                  per-head ones column (Z accumulator)
  z       2 x [128, 2048] f16     local attention output [d, t]
  zj      2 x [128, 2048] f16     jump-term accumulators (shared across g)
"""
import sys

sys.path.insert(0, "/opt/trn_rl_repo")

import numpy as np
import ml_dtypes
import concourse.bass as bass
import concourse.bacc as bacc
import concourse.tile as tile
from concourse import mybir

f32 = mybir.dt.float32
f16 = mybir.dt.float16
bf16 = mybir.dt.bfloat16
AL = mybir.AluOpType
AF = mybir.ActivationFunctionType

B, T, D = 2, 2048, 1024
H, W, E = 16, 16, 8
HD = D // H                  # 64
NCORES = 8
HPC = 4                      # heads per core
CH = HPC * HD                # 256 channels per core
NG = 2                       # head-pairs per core, 128 channels each
TP = W - 1                   # 15 (left pad)
TPAD = T + TP                # 2063
NT128 = T // 128             # 16
NT256 = T // 256             # 8
NT512 = T // 512             # 4
KT = D // 128                # 8 k-tiles over the input dim
SCALE = 1.0 / float(np.sqrt(HD))
MASKVAL = -200.0

_CACHE = {}


def build_program():
    if "nc" in _CACHE:
        return _CACHE["nc"]
    import contextlib
    nc = bacc.Bacc()

    xT = nc.dram_tensor("xT", [D, TPAD], f16, kind="ExternalInput")
    # weights packed [g, ktile, in-ch-in-tile, (tap, out-ch)] for 4KB DMA lines
    qw = nc.dram_tensor("qw", [NG, KT, 128, W * 128], f16, kind="ExternalInput")
    kw = nc.dram_tensor("kw", [NG, KT, 128, W * 128], f16, kind="ExternalInput")
    vw = nc.dram_tensor("vw", [D, CH], f16, kind="ExternalInput")
    pw = nc.dram_tensor("pw", [CH, D], f16, kind="ExternalInput")
    qb = nc.dram_tensor("qb", [CH, 1], f32, kind="ExternalInput")
    kb = nc.dram_tensor("kb", [CH, 1], f32, kind="ExternalInput")
    vb = nc.dram_tensor("vb", [CH, 1], f32, kind="ExternalInput")
    mask = nc.dram_tensor("mask", [272, 256], f32, kind="ExternalInput")
    ident = nc.dram_tensor("ident", [128, 128], f16, kind="ExternalInput")
    onesp = nc.dram_tensor("onesp", [E, 128, 2 * E], f16, kind="ExternalInput")
    ones4 = nc.dram_tensor("ones4", [128, 2 * HPC], bf16, kind="ExternalInput")
    zpad = nc.dram_tensor("zpad", [128, TP], f16, kind="ExternalInput")
    vbrow = nc.dram_tensor("vbrow", [1, CH], bf16, kind="ExternalInput")
    vzero = nc.dram_tensor("vzero", [TP, CH], bf16, kind="ExternalInput")
    y = nc.dram_tensor("y", [D, T], f16, kind="ExternalOutput")
    alpha_d = [nc.dram_tensor(f"alpha_d{g}", [2 * E, T], f16) for g in range(NG)]

    with tile.TileContext(nc) as tc:
        with contextlib.ExitStack() as ctx:
            consts = ctx.enter_context(tc.tile_pool(name="consts", bufs=1))
            main = ctx.enter_context(tc.tile_pool(name="main", bufs=1))

            # ---- x and v-weights first: the PE's first work needs them ----
            xT_sb = [main.tile([128, TPAD], f16, tag=f"x{i}", name=f"xT_sb{i}") for i in range(KT)]
            vw_sb = [consts.tile([128, CH], f16, tag=f"vw{i}", name=f"vw_sb{i}") for i in range(KT)]
            for i in range(KT):
                if i % 2 == 0:
                    nc.sync.dma_start(xT_sb[i][:], xT[128 * i:128 * (i + 1), :])
                else:
                    nc.scalar.dma_start(xT_sb[i][:], xT[128 * i:128 * (i + 1), :])
                nc.sync.dma_start(vw_sb[i][:], vw[128 * i:128 * (i + 1), :])
            vbt = consts.tile([128, CH], bf16)
            nc.sync.dma_start(vbt[:], vbrow[:].to_broadcast((128, CH)))

            # ---- remaining constants ----
            m0 = consts.tile([128, 256], f32)
            m1 = consts.tile([128, 256], f32)
            m2 = consts.tile([TP, 256], f32)
            nc.sync.dma_start(m0[:], mask[0:128, :])
            nc.sync.dma_start(m1[:], mask[128:256, :])
            nc.sync.dma_start(m2[:], mask[256:271, :])
            id_sb = consts.tile([128, 128], f16)
            nc.sync.dma_start(id_sb[:], ident[:])
            onesp_sb = consts.tile([128, E, 2 * E], f16)
            nc.sync.dma_start(onesp_sb[:], onesp.rearrange("e p m -> p e m"))
            qb_sb = consts.tile([128, NG], f32)
            kb_sb = consts.tile([128, NG], f32)
            vb_sb = consts.tile([128, NG], f32)
            nc.sync.dma_start(qb_sb[:], qb.rearrange("(g p) o -> p (g o)", g=NG))
            nc.sync.dma_start(kb_sb[:], kb.rearrange("(g p) o -> p (g o)", g=NG))
            nc.sync.dma_start(vb_sb[:], vb.rearrange("(g p) o -> p (g o)", g=NG))
            pw_sb = [consts.tile([128, D], f16, tag=f"pw{g}", name=f"pw_sb{g}") for g in range(NG)]
            for g in range(NG):
                nc.sync.dma_start(pw_sb[g][:], pw[128 * g:128 * (g + 1), :])
            q_sb = [main.tile([128, T], f16, tag=f"q{g}", name=f"q_sb{g}") for g in range(NG)]
            k_sb = [main.tile([128, TPAD], f16, tag=f"k{g}", name=f"k_sb{g}") for g in range(NG)]
            v_dt = [main.tile([128, T], f16, tag=f"vdt{g}", name=f"v_dt{g}") for g in range(NG)]
            v_td = [main.tile([128, HPC, HD + 2], bf16, tag=f"vtd{j}", name=f"v_td{j}")
                    for j in range(NT128 + 1)]
            z = [main.tile([128, T], f16, tag=f"z{g}", name=f"z{g}") for g in range(NG)]
            zj_sh = [main.tile([128, T], f16, tag=f"zj{a}", name=f"zj{a}")
                     for a in range(2)]
            zj = [zj_sh, zj_sh]          # per-g lifetimes do not overlap
            zr = [main.tile([128, T], f16, tag=f"zr{g}", name=f"zr{g}") for g in range(NG)]
            arows_sh = main.tile([2 * E, T], f16, tag="ar", name="arows")
            arows = [arows_sh, arows_sh]

            for g in range(NG):
                nc.sync.dma_start(k_sb[g][:, 0:TP], zpad[:])

            # ================= phase 1: v projections =================
            with tc.tile_pool(name="psV", bufs=1, space="PSUM") as psV:
                # v in [t, d] layout (M=t), shifted +15, with bias
                for j in range(NT128 + 1):
                    mrow = 128 if j < NT128 else TP
                    pv = psV.tile([128, 256], f32, tag=f"pv{j % 2}", bufs=1, name=f"pv{j}")
                    for i in range(KT):
                        nc.tensor.matmul(
                            pv[0:mrow, :],
                            xT_sb[i][:, 128 * j:128 * j + mrow],
                            vw_sb[i][:],
                            start=(i == 0), stop=(i == KT - 1),
                        )
                    nc.vector.tensor_tensor(
                        v_td[j][0:mrow, :, 0:HD],
                        pv[0:mrow, :].rearrange("p (h d) -> p h d", h=HPC),
                        vbt[0:mrow, :].rearrange("p (h d) -> p h d", h=HPC),
                        AL.add,
                    )
                    if j == 0:
                        # keys/values at t<0 are zero AFTER bias in the reference
                        nc.sync.dma_start(
                            v_td[0][0:TP, :, 0:HD],
                            vzero.rearrange("p (h d) -> p h d", h=HPC))
                    nc.sync.dma_start(
                        v_td[j][:, :, HD:HD + 2],
                        ones4.rearrange("p (h o) -> p h o", o=2))

                # v in [d, t] layout (M=d), with bias
                for g in range(NG):
                    for t4 in range(NT512):
                        pv2 = psV.tile([128, 512], f32, tag=f"pv2{t4 % 2}", bufs=1,
                                       name=f"pv2_{g}_{t4}")
                        for i in range(KT):
                            nc.tensor.matmul(
                                pv2[:],
                                vw_sb[i][:, 128 * g:128 * (g + 1)],
                                xT_sb[i][:, TP + 512 * t4: TP + 512 * (t4 + 1)],
                                start=(i == 0), stop=(i == KT - 1),
                            )
                        nc.vector.tensor_scalar(
                            v_dt[g][:, 512 * t4:512 * (t4 + 1)], pv2[:],
                            vb_sb[:, g:g + 1], None, op0=AL.add,
                        )

            # ======= phases 2-3: per-g conv then attention, pipelined =======
            with tc.tile_pool(name="attn", bufs=1) as apool, \
                 tc.tile_pool(name="wstream", bufs=6) as wpool, \
                 tc.tile_pool(name="psC", bufs=1, space="PSUM") as psC, \
                 tc.tile_pool(name="psA", bufs=1, space="PSUM") as psA:

                erows_sh = apool.tile([2 * E, T], bf16, tag="erows",
                                      name="erows")
                erows = [erows_sh, erows_sh]

                def conv_pair(g, p):
                    # one weight stream covers two 512-token chunks; q-pass
                    # then k-pass, 2 PSUM banks total
                    for wdram, is_q in ((qw, True), (kw, False)):
                        pcs = [psC.tile([128, 512], f32, tag=f"cv{c}", bufs=1,
                                        name=f"pc{g}_{p}_{c}") for c in range(2)]
                        for i in range(KT):
                            wt = wpool.tile([128, W * 128], f16,
                                            tag=f"w{int(is_q)}", bufs=3)
                            nc.sync.dma_start(wt[:], wdram[g, i])
                            for dt in range(W):
                                first = (i == 0 and dt == 0)
                                last = (i == KT - 1 and dt == W - 1)
                                for c in range(2):
                                    c0 = 512 * (2 * p + c) + dt
                                    nc.tensor.matmul(
                                        pcs[c][:], wt[:, 128 * dt:128 * (dt + 1)],
                                        xT_sb[i][:, c0:c0 + 512],
                                        start=first, stop=last)
                        for c in range(2):
                            t0 = 512 * (2 * p + c)
                            if is_q:
                                nc.vector.tensor_scalar(
                                    q_sb[g][:, t0:t0 + 512], pcs[c][:],
                                    qb_sb[:, g:g + 1], SCALE,
                                    op0=AL.add, op1=AL.mult)
                            else:
                                nc.vector.tensor_scalar(
                                    k_sb[g][:, TP + t0:TP + t0 + 512], pcs[c][:],
                                    kb_sb[:, g:g + 1], None, op0=AL.add)

                def jump_quarter(g, q4):
                    pj = psA.tile([2 * E, 512], f32, tag="pj", bufs=1, name="pj")
                    for e in range(E):
                        sh = 1 << e
                        t0 = 512 * q4
                        tmp = apool.tile([128, 512], f16, tag="jtmp", bufs=3,
                                         name="jtmp")
                        ov = max(0, t0 + 512 + sh - T)  # wrap amount
                        nc.vector.tensor_tensor(
                            tmp[:, 0:512 - ov],
                            q_sb[g][:, t0:t0 + 512 - ov],
                            k_sb[g][:, TP + t0 + sh:TP + t0 + sh + 512 - ov],
                            AL.mult)
                        if ov:
                            nc.vector.tensor_tensor(
                                tmp[:, 512 - ov:512],
                                q_sb[g][:, t0 + 512 - ov:t0 + 512],
                                k_sb[g][:, TP:TP + ov], AL.mult)
                        nc.tensor.matmul(
                            pj[:], onesp_sb[:, e, :], tmp[:],
                            start=(e == 0), stop=(e == E - 1))
                    nc.scalar.activation(
                        erows[g][:, 512 * q4:512 * (q4 + 1)], pj[:], AF.Exp)

                def jump_finish(g):
                    zsum = apool.tile([2 * E, 1], f32, tag="zsum", name="zsum")
                    nc.vector.tensor_reduce(
                        zsum[:], erows[g][:], mybir.AxisListType.X, AL.add)
                    nc.vector.reciprocal(zsum[:], zsum[:])
                    nc.vector.tensor_scalar(
                        arows[g][:], erows[g][:], zsum[:], None, op0=AL.mult)
                    nc.sync.dma_start(alpha_d[g][:], arows[g][:])

                def jump_fma(g):
                    # alpha_e * v_shifted accumulated elementwise on two
                    # engine lanes.  DVE/GpSimd f16 ops with non-256B-aligned
                    # slices run ~4x slow, so shifted v copies are built with
                    # SBUF->SBUF DMA and all engine ops are full-width.
                    # g0's FMA runs under conv g1: route it to GpSimd so the
                    # DVE queue stays clear for conv PSUM drains.  g1's FMA is
                    # the tail: DVE-heavy.
                    # GpSimd runs f16 elementwise at ~5us/op whether aligned
                    # or not, so only DVE lanes get the shift-aligned v copy.
                    gp_lanes = (0, 1, 2, 3, 4, 5, 6, 7) if g == 0 else (1, 3)
                    acc = zj[g]          # [lane0 accumulator, lane1 accumulator]
                    first = [True, True]
                    # g0's FMA DMAs go on the ACT queue (sync carries conv g1's
                    # weight stream); g1's go on sync (idle in the tail) so the
                    # ACT queue stays clear for local-attention exps.
                    dma_eng = nc.scalar if g == 0 else nc.sync
                    for e in range(E):
                        sh = 1 << e
                        lane = e % 2
                        on_gp = e in gp_lanes
                        eng = nc.gpsimd if on_gp else nc.vector
                        bc = apool.tile([128, T], f16, tag="bc", bufs=3, name="bc")
                        dma_eng.dma_start(
                            bc[0:64, :],
                            alpha_d[g][2 * e:2 * e + 1, :].to_broadcast((64, T)))
                        dma_eng.dma_start(
                            bc[64:128, :],
                            alpha_d[g][2 * e + 1:2 * e + 2, :].to_broadcast((64, T)))
                        aligned = not (sh % 128) or on_gp
                        if not aligned:
                            vsh = apool.tile([128, T], f16, tag="vsh", bufs=3,
                                             name="vsh")
                            dma_eng.dma_start(vsh[:, 0:T - sh], v_dt[g][:, sh:T])
                            dma_eng.dma_start(vsh[:, T - sh:T], v_dt[g][:, 0:sh])
                        if first[lane]:
                            ft = acc[lane]
                        else:
                            ft = apool.tile([128, T], f16, tag=f"ft{lane}",
                                            bufs=2, name="ft")
                        if not aligned:
                            eng.tensor_tensor(ft[:], bc[:], vsh[:], AL.mult)
                        else:
                            eng.tensor_tensor(
                                ft[:, 0:T - sh], bc[:, 0:T - sh],
                                v_dt[g][:, sh:T], AL.mult)
                            eng.tensor_tensor(
                                ft[:, T - sh:T], bc[:, T - sh:T],
                                v_dt[g][:, 0:sh], AL.mult)
                        if first[lane]:
                            first[lane] = False
                        else:
                            eng.tensor_tensor(
                                acc[lane][:], acc[lane][:], ft[:], AL.add)

                def scores_part(g, c):
                    exps = []
                    for hh in range(2):
                        r0, r1 = 64 * hh, 64 * hh + 64
                        e0 = apool.tile([128, 256], bf16, tag="e0", bufs=3, name="e0")
                        e1 = apool.tile([128, 256], bf16, tag="e1", bufs=3, name="e1")
                        e2 = apool.tile([TP, 256], bf16, tag="e2", bufs=3, name="e2")
                        for (et, msk, s0, srows) in (
                            (e0, m0, 256 * c, 128),
                            (e1, m1, 256 * c + 128, 128),
                            (e2, m2, 256 * c + 256, TP),
                        ):
                            ps = psA.tile([128, 256], f32, tag="score", bufs=2, name="ps")
                            nc.tensor.matmul(
                                ps[0:srows, :],
                                k_sb[g][r0:r1, s0:s0 + srows],
                                q_sb[g][r0:r1, 256 * c:256 * (c + 1)],
                                start=True, stop=True,
                            )
                            nc.vector.tensor_tensor(
                                ps[0:srows, :], ps[0:srows, :],
                                msk[0:srows, :], AL.add)
                            nc.scalar.activation(
                                et[0:srows, :], ps[0:srows, :], AF.Exp)
                        exps.append((e0, e1, e2))
                    return exps

                def po_part(g, c, exps):
                    for sub in range(2):
                            jj = 2 * c + sub
                            stage = apool.tile([128, 128], f16, tag="stage", bufs=2, name="stage")
                            for hh in range(2):
                                e0, e1, e2 = exps[hh]
                                if sub == 0:
                                    lo, hi = e0[:, 0:128], e1[0:TP, 0:128]
                                else:
                                    lo, hi = e1[:, 128:256], e2[0:TP, 128:256]
                                hl = 2 * g + hh
                                po = psA.tile([128, HD + 2], f32, tag="popt", bufs=2, name="po")
                                nc.tensor.matmul(
                                    po[:], lo, v_td[jj][:, hl, :],
                                    start=True, stop=False,
                                )
                                nc.tensor.matmul(
                                    po[:], hi, v_td[jj + 1][0:TP, hl, :],
                                    start=False, stop=True,
                                )
                                rz = apool.tile([128, 1], f32, tag="rz", bufs=2, name="rz")
                                nc.vector.reciprocal(rz[:], po[:, HD:HD + 1])
                                nc.vector.tensor_scalar(
                                    stage[:, 64 * hh:64 * hh + 64],
                                    po[:, 0:HD], rz[:], None, op0=AL.mult,
                                )
                            pt = psA.tile([128, 128], f16, tag="popt", bufs=2, name="pt")
                            nc.tensor.transpose(pt[:], stage[:], id_sb[:])
                            tcol = 256 * c + 128 * sub
                            nc.vector.tensor_copy(z[g][:, tcol:tcol + 128], pt[:])

                def local_attn(g, chunks):
                    for c in chunks:
                        po_part(g, c, scores_part(g, c))

                for g in range(NG):
                    conv_pair(g, 0)              # tokens 0..1023
                    local_attn(g, [0, 1, 2, 3])  # keys <= t1023 available
                    jump_quarter(g, 0)           # k up to t640 available
                    conv_pair(g, 1)              # tokens 1024..2047
                    jump_quarter(g, 1)
                    jump_quarter(g, 2)
                    jump_quarter(g, 3)
                    jump_finish(g)
                    jump_fma(g)
                    local_attn(g, [4, 5, 6, 7])
                    nc.vector.tensor_tensor(zr[g][:], z[g][:], zj[g][0][:], AL.add)
                    nc.vector.tensor_tensor(zr[g][:], zr[g][:], zj[g][1][:], AL.add)

            # ================= phase 4: output projection =================
            with tc.tile_pool(name="proj", bufs=1) as ppool, \
                 tc.tile_pool(name="psP", bufs=1, space="PSUM") as psP:
                for o8 in range(D // 128):
                    for t4 in range(NT512):
                        py = psP.tile([128, 512], f32, tag="py", bufs=4, name="py")
                        for g in range(NG):
                            nc.tensor.matmul(
                                py[:],
                                pw_sb[g][:, 128 * o8:128 * (o8 + 1)],
                                zr[g][:, 512 * t4:512 * (t4 + 1)],
                                start=(g == 0), stop=(g == NG - 1),
                            )
                        ysb = ppool.tile([128, 512], f16, tag="ysb", bufs=4, name="ysb")
                        nc.vector.tensor_copy(ysb[:], py[:])
                        eng = nc.sync if (o8 * NT512 + t4) % 2 == 0 else nc.scalar
                        eng.dma_start(
                            y[128 * o8:128 * (o8 + 1),
                              512 * t4:512 * (t4 + 1)], ysb[:])

    nc.compile()
    _CACHE["nc"] = nc
    return nc


def make_consts():
    mask = np.full((272, 256), MASKVAL, np.float32)
    rel = np.arange(271)[:, None]
    trel = np.arange(256)[None, :]
    band = (rel >= trel) & (rel <= trel + TP)
    mask[:271][band] = 0.0
    ident = np.eye(128, dtype=np.float16)
    onesp = np.zeros((E, 128, 2 * E), np.float16)
    for e in range(E):
        onesp[e, 0:64, 2 * e] = 1.0
        onesp[e, 64:128, 2 * e + 1] = 1.0
    ones4 = np.zeros((128, 2 * HPC), ml_dtypes.bfloat16)
    ones4[:, 0::2] = 1.0
    zpad = np.zeros((128, TP), np.float16)
    return mask, ident, onesp, ones4, zpad


def _pack_conv_w(wslice):
    # wslice [CH, D, W] (torch layout for this core's channels) ->
    # [NG, KT, 128, W*128]: [g, i, in-ch p, (tap dt, out-ch o)]
    a = wslice.reshape(NG, 128, KT, 128, W)         # [g, o, i, p, dt]
    a = a.transpose(0, 2, 3, 4, 1)                  # [g, i, p, dt, o]
    return np.ascontiguousarray(
        a.reshape(NG, KT, 128, W * 128).astype(np.float16))


def make_in_maps(x, q_w, q_b, k_w, k_b, v_w, v_b, p_w):
    mask, ident, onesp, ones4, zpad = make_consts()
    in_maps = []
    for core in range(NCORES):
        b, g = core // HPC, core % HPC
        ch = slice(CH * g, CH * (g + 1))
        xTf = np.zeros((D, TPAD), np.float16)
        xTf[:, TP:] = x[b].T.astype(np.float16)
        in_maps.append({
            "xT": xTf,
            "qw": _pack_conv_w(q_w[ch]),
            "kw": _pack_conv_w(k_w[ch]),
            "vw": np.ascontiguousarray(v_w[ch].T.astype(np.float16)),
            "pw": np.ascontiguousarray(p_w[:, ch].T.astype(np.float16)),
            "qb": np.ascontiguousarray(q_b[ch][:, None].astype(np.float32)),
            "kb": np.ascontiguousarray(k_b[ch][:, None].astype(np.float32)),
            "vb": np.ascontiguousarray(v_b[ch][:, None].astype(np.float32)),
            "mask": mask, "ident": ident, "onesp": onesp,
            "ones4": ones4, "zpad": zpad,
            "vbrow": np.ascontiguousarray(
                v_b[ch][None, :].astype(ml_dtypes.bfloat16)),
            "vzero": np.zeros((TP, CH), ml_dtypes.bfloat16),
        })
    return in_maps


def assemble_output(results, p_b):
    out = np.zeros((B, T, D), np.float32)
    for core in range(NCORES):
        out[core // HPC] += results[core]["y"].T.astype(np.float32)
    out += p_b[None, None, :]
    return out


def _run(inputs, trace=False):
    from concourse.bass_utils import run_bass_kernel_spmd
    nc = build_program()
    args = {k: np.asarray(v, np.float32) for k, v in inputs.items()}
    p_b = args.pop("p_b")
    in_maps = make_in_maps(**args)
    res = run_bass_kernel_spmd(nc, in_maps, list(range(NCORES)), trace=trace)
    out = assemble_output(res.results, p_b)
    return out, res


def kernel(**inputs):
    out, _ = _run(inputs)
    return out
